# revision 1
# baseline (speedup 1.0000x reference)
"""Masked-reconstruction (stem->conv->GRU->head->masked MSE) Bass kernel.

Layouts (per core, B_C=8 batch rows):
  xT   (8, 64, T) f32   - x transposed to feature-major
  keepT(8, 64, T) u8    - 1 - feature_mask (encoder keep mask)
  mT   (8, 64, T) u8    - feature_mask (loss mask)
  convh(8, 64, T) f32   - DRAM staging of conv output
  zseq (128, 8, T) f32  - DRAM staging of GRU hidden sequence
  out  (64, 4) f32      - per-feature partials [Sf, sum_x, sum_x2, sum_m]

Scan: hidden-on-partitions (128, 8 batch). gx for S=64 steps prefilled into
PSUM by PE (bias baked via ones-row, K=65). Per step: 3 recurrent matmuls
accumulate, one sigmoid ACT for [r|z], scalar_tensor_tensor + add for n-gate
pre-activation, tanh ACT, 3 DVE ops for the h update.
"""
from contextlib import ExitStack

import numpy as np

import concourse.bass as bass
import concourse.mybir as mybir
import concourse.tile as tile
from concourse import bacc
from concourse.bass import ts

F32 = mybir.dt.float32
U8 = mybir.dt.uint8
AF = mybir.ActivationFunctionType
ALU = mybir.AluOpType

B, F, DH, DG = 64, 64, 64, 128
NCORE = 8
B_C = B // NCORE  # 8 batch rows per core
S = 64            # GRU steps per chunk
TILE_T = 512      # encoder/head tile width


def prep_inputs(inputs, T):
    """Host-side layout prep. Returns (shared weight dict, per-core input dicts)."""
    x = np.asarray(inputs["x"], np.float32)
    fm = np.asarray(inputs["feature_mask"])
    w = {}
    w["stemW"] = np.ascontiguousarray(np.asarray(inputs["stem_w"], np.float32))  # (F, DH) = lhsT
    w["stemB"] = np.asarray(inputs["stem_b"], np.float32).reshape(DH, 1)
    cw = np.asarray(inputs["conv_w"], np.float32)  # (DH_out, DH_in, 3)
    w["convW"] = np.ascontiguousarray(cw.transpose(2, 1, 0))  # (3, in, out) lhsT per dt
    w["convB"] = np.asarray(inputs["conv_b"], np.float32).reshape(DH, 1)
    wih = np.asarray(inputs["gru_w_ih"], np.float32)  # (3DG, DH)
    whh = np.asarray(inputs["gru_w_hh"], np.float32)  # (3DG, DG)
    bih = np.asarray(inputs["gru_b_ih"], np.float32)
    bhh = np.asarray(inputs["gru_b_hh"], np.float32)
    wihT = np.zeros((3, DH + 1, DG), np.float32)
    for g in range(3):
        wihT[g, :DH] = wih[g * DG:(g + 1) * DG].T
        bias = bih[g * DG:(g + 1) * DG].copy()
        if g < 2:  # r,z: fold b_hh into the prefilled bias
            bias += bhh[g * DG:(g + 1) * DG]
        wihT[g, DH] = bias
    w["wihT"] = wihT
    w["whhT"] = np.ascontiguousarray(np.stack([whh[g * DG:(g + 1) * DG].T for g in range(3)]))
    w["bhhn"] = bhh[2 * DG:].reshape(DG, 1).copy()
    w["h1w"] = np.ascontiguousarray(np.asarray(inputs["h1_w"], np.float32))  # (DG,128) lhsT
    w["h1b"] = np.asarray(inputs["h1_b"], np.float32).reshape(128, 1)
    w["h2w"] = np.ascontiguousarray(np.asarray(inputs["h2_w"], np.float32))
    w["h2b"] = np.asarray(inputs["h2_b"], np.float32).reshape(128, 1)
    w["h3w"] = np.ascontiguousarray(np.asarray(inputs["h3_w"], np.float32))  # (128, F) lhsT
    w["h3b"] = np.asarray(inputs["h3_b"], np.float32).reshape(F, 1)

    per_core = []
    for c in range(NCORE):
        rows = slice(c * B_C, (c + 1) * B_C)
        xc = np.ascontiguousarray(x[rows].transpose(0, 2, 1))        # (B_C, F, T)
        fmc = fm[rows].transpose(0, 2, 1)
        d = dict(w)
        d["xT"] = xc
        d["keepT"] = np.ascontiguousarray((~fmc).astype(np.uint8))
        d["mT"] = np.ascontiguousarray(fmc.astype(np.uint8))
        per_core.append(d)
    return per_core


def host_finalize(core_outs, T):
    """Combine per-core (64,4) partials into the scalar loss."""
    tot = np.sum([np.asarray(o, np.float64) for o in core_outs], axis=0)  # (64,4)
    sf, sx, sx2, sm = tot[:, 0], tot[:, 1], tot[:, 2], tot[:, 3]
    n = B * T
    var = (sx2 - sx * sx / n) / (n - 1)
    scale = np.sqrt(np.maximum(var, 0.0)) + 1e-8
    num = np.sum(sf / (scale * scale))
    den = max(sm.sum(), 1.0)
    return np.float32(num / den)


def build_program(T, phases="abc", scan_repeats=1):
    assert T % TILE_T == 0 and T % S == 0
    NT = T // TILE_T          # encoder/head tiles per row
    NCH = T // S              # scan chunks
    nc = bacc.Bacc("TRN2", target_bir_lowering=False, debug=False,
                   num_devices=NCORE)

    # ---- DRAM tensors ----
    xT = nc.dram_tensor("xT", [B_C, F, T], F32, kind="ExternalInput").ap()
    keepT = nc.dram_tensor("keepT", [B_C, F, T], U8, kind="ExternalInput").ap()
    mT = nc.dram_tensor("mT", [B_C, F, T], U8, kind="ExternalInput").ap()
    stemW = nc.dram_tensor("stemW", [F, DH], F32, kind="ExternalInput").ap()
    stemB = nc.dram_tensor("stemB", [DH, 1], F32, kind="ExternalInput").ap()
    convW = nc.dram_tensor("convW", [3, DH, DH], F32, kind="ExternalInput").ap()
    convB = nc.dram_tensor("convB", [DH, 1], F32, kind="ExternalInput").ap()
    wihT = nc.dram_tensor("wihT", [3, DH + 1, DG], F32, kind="ExternalInput").ap()
    whhT = nc.dram_tensor("whhT", [3, DG, DG], F32, kind="ExternalInput").ap()
    bhhn = nc.dram_tensor("bhhn", [DG, 1], F32, kind="ExternalInput").ap()
    h1w = nc.dram_tensor("h1w", [DG, 128], F32, kind="ExternalInput").ap()
    h1b = nc.dram_tensor("h1b", [128, 1], F32, kind="ExternalInput").ap()
    h2w = nc.dram_tensor("h2w", [128, 128], F32, kind="ExternalInput").ap()
    h2b = nc.dram_tensor("h2b", [128, 1], F32, kind="ExternalInput").ap()
    h3w = nc.dram_tensor("h3w", [128, F], F32, kind="ExternalInput").ap()
    h3b = nc.dram_tensor("h3b", [F, 1], F32, kind="ExternalInput").ap()
    convh = nc.dram_tensor("convh", [B_C, DH, T], F32).ap()
    zseq = nc.dram_tensor("zseq", [DG, B_C, T], F32).ap()
    out = nc.dram_tensor("out", [64, 4], F32, kind="ExternalOutput").ap()

    with tile.TileContext(nc) as tc, ExitStack() as ctx:
        # ---- persistent pools ----
        wpool = ctx.enter_context(tc.tile_pool(name="weights", bufs=1))
        spool = ctx.enter_context(tc.tile_pool(name="stats", bufs=1))

        def wtile(shape, src, tag):
            t = wpool.tile(shape, F32, tag=tag)
            nc.sync.dma_start(t[:], src)
            return t

        stemW_s = wtile([F, DH], stemW[:], "w_stemW")
        stemB_s = wtile([DH, 1], stemB[:], "w_stemB")
        convW_s = [wtile([DH, DH], convW[dt], f"w_convW{dt}") for dt in range(3)]
        convB_s = wtile([DH, 1], convB[:], "w_convB")
        wih_s = [wtile([DH + 1, DG], wihT[g], f"w_wih{g}") for g in range(3)]
        whh_s = [wtile([DG, DG], whhT[g], f"w_whh{g}") for g in range(3)]
        bhhn_s = wtile([DG, 1], bhhn[:], "w_bhhn")
        h1w_s = wtile([DG, 128], h1w[:], "w_h1w")
        h1b_s = wtile([128, 1], h1b[:], "w_h1b")
        h2w_s = wtile([128, 128], h2w[:], "w_h2w")
        h2b_s = wtile([128, 1], h2b[:], "w_h2b")
        h3w_s = wtile([128, F], h3w[:], "w_h3w")
        h3b_s = wtile([F, 1], h3b[:], "w_h3b")

        zeros8 = wpool.tile([DG, B_C], F32)
        nc.vector.memset(zeros8[:], 0.0)

        # stats partials, one column per (b, tile)
        NPART = B_C * NT
        st_sf = spool.tile([64, NPART], F32)
        st_sx = spool.tile([64, NPART], F32)
        st_sx2 = spool.tile([64, NPART], F32)
        st_sm = spool.tile([64, NPART], F32)

        # ================= Phase A: encoder =================
        if "a" in phases:
         with tc.tile_pool(name="enc_io", bufs=3) as io, \
             tc.tile_pool(name="enc_row", bufs=2) as rowp, \
             tc.tile_pool(name="enc_ps", bufs=2, space="PSUM") as eps, \
             tc.tile_pool(name="enc_tmp", bufs=3) as etmp:
            for b in range(B_C):
                hrow = rowp.tile([DH, T + 2], F32, tag="hrow")
                nc.vector.memset(hrow[:, 0:1], 0.0)
                nc.vector.memset(hrow[:, T + 1:T + 2], 0.0)
                for it in range(NT):
                    col = b * NT + it
                    tsl = ts(it, TILE_T)
                    xt = io.tile([F, TILE_T], F32, tag="xt")
                    nc.sync.dma_start(xt[:], xT[b, :, tsl])
                    ku = io.tile([F, TILE_T], U8, tag="ku")
                    nc.sync.dma_start(ku[:], keepT[b, :, tsl])
                    kf = etmp.tile([F, TILE_T], F32, tag="kf")
                    nc.vector.tensor_copy(kf[:], ku[:])
                    xm = etmp.tile([F, TILE_T], F32, tag="xm")
                    nc.vector.tensor_mul(xm[:], xt[:], kf[:])
                    # x stats
                    sq = etmp.tile([F, TILE_T], F32, tag="sq")
                    nc.scalar.activation(sq[:], xt[:], AF.Square,
                                         accum_out=st_sx2[:, col:col + 1])
                    nc.vector.tensor_reduce(st_sx[:, col:col + 1], xt[:],
                                            mybir.AxisListType.X, ALU.add)
                    # stem
                    ps = eps.tile([DH, TILE_T], F32, tag="stem_ps")
                    nc.tensor.matmul(ps[:], stemW_s[:], xm[:], start=True, stop=True)
                    nc.scalar.activation(hrow[:, 1 + it * TILE_T:1 + (it + 1) * TILE_T],
                                         ps[:], AF.Gelu, bias=stemB_s[:])
                # conv over the padded row
                for it in range(NT):
                    ps = eps.tile([DH, TILE_T], F32, tag="conv_ps")
                    for dt in range(3):
                        nc.tensor.matmul(ps[:], convW_s[dt][:],
                                         hrow[:, it * TILE_T + dt:it * TILE_T + dt + TILE_T],
                                         start=(dt == 0), stop=(dt == 2))
                    ch = etmp.tile([DH, TILE_T], F32, tag="ch")
                    nc.scalar.activation(ch[:], ps[:], AF.Gelu, bias=convB_s[:])
                    nc.sync.dma_start(convh[b, :, ts(it, TILE_T)], ch[:])

        # ================= Phase B: GRU scan =================
        # h_t = u_t + v_t with u_t = (1-z_t)*n_t, v_t = z_t*h_{t-1}.
        # The recurrent matmuls take u and v as two moving operands, so the
        # only post-tanh op on the critical path is u = z' * n.
        if "b" in phases:
         with tc.tile_pool(name="scan_in", bufs=3) as sin, \
             tc.tile_pool(name="scan_h", bufs=2) as shp, \
             tc.tile_pool(name="scan_gx", bufs=2) as sgx, \
             tc.tile_pool(name="scan_sm", bufs=4) as ssm, \
             tc.tile_pool(name="ps_rz", bufs=2, space="PSUM") as prz, \
             tc.tile_pool(name="ps_n", bufs=2, space="PSUM") as pn, \
             tc.tile_pool(name="ps_s", bufs=2, space="PSUM") as pscr:
            # u/v of the step before this chunk; v=None means h comes from
            # the single tensor u (zeros at t=0).
            prev_u, prev_v, prev_hmat = zeros8, None, zeros8
            cins = {}

            def load_cin_force(k):
                cins.pop(k, None)
                load_cin(k)

            def load_cin(k):
                if k >= NCH or k in cins:
                    return
                cin = sin.tile([DH + 1, B_C, S], F32, tag="cin")
                nc.sync.dma_start(
                    cin[0:DH], convh[:, :, ts(k, S)].rearrange("b c t -> c b t"))
                nc.vector.memset(cin[DH:DH + 1], 1.0)
                cins[k] = cin

            load_cin(0)
            for rep in range(scan_repeats):
             for k in range(NCH):
                load_cin(k + 1)
                cin = cins.pop(k)
                if rep + 1 < scan_repeats and k + 1 == NCH:
                    load_cin_force(0)
                P_rz = prz.tile([DG, 2, B_C, S], F32, tag="prz")
                P_n = pn.tile([DG, B_C, S], F32, tag="pn")
                nc.tensor.matmul(P_rz[:, 0], wih_s[0][:], cin[:], start=True,
                                 stop=False, skip_group_check=True)
                nc.tensor.matmul(P_rz[:, 1], wih_s[1][:], cin[:], start=True,
                                 stop=False, skip_group_check=True)
                nc.tensor.matmul(P_n[:], wih_s[2][:], cin[:], start=True,
                                 stop=True, skip_group_check=True)
                gxn = sgx.tile([DG, B_C, S], F32, tag="gxn")
                nc.scalar.copy(gxn[:], P_n[:])
                hout = shp.tile([DG, B_C, S], F32, tag="hout")
                P_s_next = None
                for t in range(S):
                    # u-part of gh for slice t (v-part was issued at step t-1;
                    # at a chunk boundary the previous v is applied here too)
                    boundary_v = prev_v if t == 0 else None
                    nc.tensor.matmul(P_rz[:, 0, :, t], whh_s[0][:], prev_u[:],
                                     start=False, stop=True, skip_group_check=True)
                    if boundary_v is not None:
                        nc.tensor.matmul(P_rz[:, 0, :, t], whh_s[0][:], boundary_v[:],
                                         start=False, stop=True, skip_group_check=True)
                    r_sb = ssm.tile([DG, B_C], F32, tag="r_sb")
                    nc.scalar.activation(r_sb[:], P_rz[:, 0, :, t], AF.Sigmoid)
                    nc.tensor.matmul(P_rz[:, 1, :, t], whh_s[1][:], prev_u[:],
                                     start=False, stop=True, skip_group_check=True)
                    if boundary_v is not None:
                        nc.tensor.matmul(P_rz[:, 1, :, t], whh_s[1][:], boundary_v[:],
                                         start=False, stop=True, skip_group_check=True)
                    if P_s_next is None:
                        P_s = pscr.tile([DG, B_C], F32, tag="pscr")
                        nc.tensor.matmul(P_s[:], whh_s[2][:], prev_u[:],
                                         start=True, stop=True, skip_group_check=True)
                        if boundary_v is not None:
                            nc.tensor.matmul(P_s[:], whh_s[2][:], boundary_v[:],
                                             start=False, stop=True,
                                             skip_group_check=True)
                    else:
                        P_s = P_s_next
                        nc.tensor.matmul(P_s[:], whh_s[2][:], prev_u[:],
                                         start=False, stop=True, skip_group_check=True)
                    z_sb = ssm.tile([DG, B_C], F32, tag="z_sb")
                    nc.scalar.activation(z_sb[:], P_rz[:, 1, :, t], AF.Sigmoid)
                    zp_sb = ssm.tile([DG, B_C], F32, tag="zp_sb")
                    nc.scalar.activation(zp_sb[:], P_rz[:, 1, :, t], AF.Sigmoid,
                                         scale=-1.0)
                    # v_t = z_t * h_{t-1} (h materialized into hout at t-1)
                    v_sb = ssm.tile([DG, B_C], F32, tag="v_sb")
                    nc.vector.tensor_mul(v_sb[:], z_sb[:], prev_hmat[:])
                    # early v-matmuls for slice t+1 (same chunk only)
                    if t + 1 < S:
                        nc.tensor.matmul(P_rz[:, 0, :, t + 1], whh_s[0][:], v_sb[:],
                                         start=False, stop=True, skip_group_check=True)
                        nc.tensor.matmul(P_rz[:, 1, :, t + 1], whh_s[1][:], v_sb[:],
                                         start=False, stop=True, skip_group_check=True)
                        P_s_next = pscr.tile([DG, B_C], F32, tag="pscr")
                        nc.tensor.matmul(P_s_next[:], whh_s[2][:], v_sb[:],
                                         start=True, stop=True, skip_group_check=True)
                    else:
                        P_s_next = None
                    # ghn+bhhn to SBUF off-path (DVE), then the r-product and
                    # gxn-add run on GPSIMD back-to-back (no inter-op ack).
                    ghn = ssm.tile([DG, B_C], F32, tag="ghn")
                    nc.vector.tensor_scalar(ghn[:], P_s[:], bhhn_s[:], None,
                                            ALU.add)
                    tmp = ssm.tile([DG, B_C], F32, tag="tmp")
                    nc.gpsimd.tensor_tensor(tmp[:], ghn[:], r_sb[:], ALU.mult)
                    narg = ssm.tile([DG, B_C], F32, tag="narg")
                    nc.gpsimd.tensor_tensor(narg[:], tmp[:], gxn[:, :, t], ALU.add)
                    nt_ = ssm.tile([DG, B_C], F32, tag="nt")
                    nc.scalar.activation(nt_[:], narg[:], AF.Tanh)
                    u_sb = ssm.tile([DG, B_C], F32, tag="u_sb")
                    nc.vector.tensor_mul(u_sb[:], zp_sb[:], nt_[:])
                    nc.vector.tensor_add(hout[:, :, t], u_sb[:], v_sb[:])
                    prev_u, prev_v, prev_hmat = u_sb, v_sb, hout[:, :, t]
                nc.gpsimd.dma_start(zseq[:, :, ts(k, S)], hout[:])

        # ================= Phase C: head + loss =================
        if "c" in phases:
         with tc.tile_pool(name="head_io", bufs=3) as hio, \
             tc.tile_pool(name="head_tmp", bufs=3) as htmp, \
             tc.tile_pool(name="head_ps", bufs=2, space="PSUM") as hps:
            for b in range(B_C):
                for it in range(NT):
                    col = b * NT + it
                    tsl = ts(it, TILE_T)
                    zt = hio.tile([DG, TILE_T], F32, tag="zt")
                    nc.sync.dma_start(zt[:], zseq[:, b, tsl])
                    p1 = hps.tile([128, TILE_T], F32, tag="p1")
                    nc.tensor.matmul(p1[:], h1w_s[:], zt[:], start=True, stop=True)
                    r1 = htmp.tile([128, TILE_T], F32, tag="r1")
                    nc.scalar.activation(r1[:], p1[:], AF.Gelu, bias=h1b_s[:])
                    p2 = hps.tile([128, TILE_T], F32, tag="p2")
                    nc.tensor.matmul(p2[:], h2w_s[:], r1[:], start=True, stop=True)
                    r2 = htmp.tile([128, TILE_T], F32, tag="r2")
                    nc.scalar.activation(r2[:], p2[:], AF.Gelu, bias=h2b_s[:])
                    p3 = hps.tile([F, TILE_T], F32, tag="p3")
                    nc.tensor.matmul(p3[:], h3w_s[:], r2[:], start=True, stop=True)
                    xt = hio.tile([F, TILE_T], F32, tag="hxt")
                    nc.sync.dma_start(xt[:], xT[b, :, tsl])
                    mu = hio.tile([F, TILE_T], U8, tag="mu")
                    nc.sync.dma_start(mu[:], mT[b, :, tsl])
                    mf = htmp.tile([F, TILE_T], F32, tag="mf")
                    nc.vector.tensor_copy(mf[:], mu[:])
                    nc.vector.tensor_reduce(st_sm[:, col:col + 1], mf[:],
                                            mybir.AxisListType.X, ALU.add)
                    diff = htmp.tile([F, TILE_T], F32, tag="diff")
                    nc.vector.scalar_tensor_tensor(diff[:], p3[:], h3b_s[:],
                                                   xt[:], ALU.add, ALU.subtract)
                    d2 = htmp.tile([F, TILE_T], F32, tag="d2")
                    nc.vector.tensor_mul(d2[:], diff[:], diff[:])
                    d2m = htmp.tile([F, TILE_T], F32, tag="d2m")
                    nc.vector.tensor_mul(d2m[:], d2[:], mf[:])
                    nc.vector.tensor_reduce(st_sf[:, col:col + 1], d2m[:],
                                            mybir.AxisListType.X, ALU.add)

            ostage = htmp.tile([64, 4], F32, tag="ostage")
            nc.vector.tensor_reduce(ostage[:, 0:1], st_sf[:], mybir.AxisListType.X, ALU.add)
            nc.vector.tensor_reduce(ostage[:, 1:2], st_sx[:], mybir.AxisListType.X, ALU.add)
            nc.vector.tensor_reduce(ostage[:, 2:3], st_sx2[:], mybir.AxisListType.X, ALU.add)
            nc.vector.tensor_reduce(ostage[:, 3:4], st_sm[:], mybir.AxisListType.X, ALU.add)
            nc.sync.dma_start(out[:], ostage[:])

    nc.compile()
    return nc




_CACHE = {}


def kernel(**inputs):
    """Full-input entry point: shards over 8 NeuronCores, runs the Bass
    program, returns the scalar loss (np.float32)."""
    from concourse.bass_utils import run_bass_kernel_spmd

    T = int(np.asarray(inputs["x"]).shape[1])
    if "nc" not in _CACHE:
        _CACHE["nc"] = build_program(T)
    nc = _CACHE["nc"]
    per_core = prep_inputs(inputs, T)
    res = run_bass_kernel_spmd(nc, per_core, list(range(NCORE))).results
    return np.float32(host_finalize([r["out"] for r in res], T))



# revision 14
# speedup vs baseline: 12.0797x; 12.0797x over previous
"""Masked-reconstruction Bass kernel, v2 (instruction-count-optimized).

Per core (B_C=8 rows; 8-core data parallel over batch):

* bf16 matmul/activation path (4e-6 validated), fp32 PSUM/biases.
* Encoder 2-row packed via block-diag weights; conv output is unpacked to a
  64-partition SBUF buffer by DMA so scan gx-prefill needs ONE matmul per
  gate per 2 steps over all 512 columns.
* GRU scan windowed-parallel: C=64-step chunks, W=8 warmup from h=0
  (truncation ~3e-8). 8 rows x 64 chunks = 512 columns advance per step;
  72 sequential steps total. Two parity groups interleave to hide latency.
  h = n + z*(h - n); biases ride on ACT bias / scalar_tensor_tensor.
* Chunk 0's warmup runs on zero-padded conv data; h is mask-zeroed at
  step W which reproduces h=0 at t=0 exactly.
* Head+loss is a separate phase (keeps Gelu/Sigmoid ACT table sets from
  thrashing); consumes z straight from SBUF. Loss emits per-(parity,
  feature) masked-SSE partials; x-stats/scale/mask-count are host side.
"""
from contextlib import ExitStack

import numpy as np
import ml_dtypes

import concourse.bass as bass
import concourse.mybir as mybir
import concourse.tile as tile
from concourse import bacc
from concourse.bass import ts

F32 = mybir.dt.float32
BF16 = mybir.dt.bfloat16
AF = mybir.ActivationFunctionType
ALU = mybir.AluOpType
NPBF = ml_dtypes.bfloat16

B, F, DH, DG = 64, 64, 64, 128
NCORE = 8
B_C = B // NCORE          # 8 rows/core
NPAIR = B_C // 2          # 4 row pairs (2-row packing in encoder)
W = 8                     # GRU warmup steps
C = 64                    # GRU chunk length
BS = 2                    # prefill steps per PSUM block
JB = 2                    # head j-steps per block
XLB = 8                   # loss staging depth (j per DMA)
TILE_T = 512
USE_TTR = False           # tensor_tensor_reduce for the masked-SSE reduce
SPLIT_P3 = True           # x_recon packed 2 rows via base-64 PSUM output


def _bd(w):
    out = np.zeros((128, 128), np.float32)
    out[:64, :64] = w
    out[64:, 64:] = w
    return out


def prep_inputs(inputs, T):
    NCH = T // C
    x = np.asarray(inputs["x"], np.float32)
    fm = np.asarray(inputs["feature_mask"])
    xm = np.where(fm, 0.0, x)

    w = {}
    w["stemW2"] = _bd(np.asarray(inputs["stem_w"], np.float32)).astype(NPBF)
    w["stemB2"] = np.tile(np.asarray(inputs["stem_b"], np.float32), 2).reshape(128, 1)
    cw = np.asarray(inputs["conv_w"], np.float32)
    w["convW2"] = np.stack([_bd(cw[:, :, dt].T) for dt in range(3)]).astype(NPBF)
    w["convB2"] = np.tile(np.asarray(inputs["conv_b"], np.float32), 2).reshape(128, 1)

    wih = np.asarray(inputs["gru_w_ih"], np.float32)
    whh = np.asarray(inputs["gru_w_hh"], np.float32)
    bih = np.asarray(inputs["gru_b_ih"], np.float32)
    bhh = np.asarray(inputs["gru_b_hh"], np.float32)
    w["wihT"] = np.ascontiguousarray(
        np.stack([wih[g * DG:(g + 1) * DG].T for g in range(3)])).astype(NPBF)
    w["whhT"] = np.ascontiguousarray(
        np.stack([whh[g * DG:(g + 1) * DG].T for g in range(3)])).astype(NPBF)
    w["bR"] = (bih[:DG] + bhh[:DG]).reshape(DG, 1).copy()
    w["bZ"] = (bih[DG:2 * DG] + bhh[DG:2 * DG]).reshape(DG, 1).copy()
    w["bIN"] = bih[2 * DG:].reshape(DG, 1).copy()
    w["bHN"] = bhh[2 * DG:].reshape(DG, 1).copy()

    w["h1w"] = np.ascontiguousarray(np.asarray(inputs["h1_w"], np.float32)).astype(NPBF)
    w["h1b"] = np.asarray(inputs["h1_b"], np.float32).reshape(128, 1)
    w["h2w"] = np.ascontiguousarray(np.asarray(inputs["h2_w"], np.float32)).astype(NPBF)
    w["h2b"] = np.asarray(inputs["h2_b"], np.float32).reshape(128, 1)
    w["h3w"] = np.ascontiguousarray(np.asarray(inputs["h3_w"], np.float32)).astype(NPBF)
    w["h3b2"] = np.tile(np.asarray(inputs["h3_b"], np.float32), 2).reshape(128, 1)

    per_core = []
    for cc in range(NCORE):
        rows = slice(cc * B_C, (cc + 1) * B_C)
        xmc = xm[rows]
        xmP = (xmc.reshape(NPAIR, 2, T, F)
               .transpose(0, 1, 3, 2)
               .reshape(NPAIR, 128, T)).astype(NPBF)

        def pack_loss(a):
            # [64q+f, j, (p, c)] with col order matching scan columns
            return np.ascontiguousarray(
                a[rows].reshape(NPAIR, 2, NCH, C, F)      # (p,q,c,j,f)
                .transpose(1, 4, 3, 0, 2)                 # (q,f,j,p,c)
                .reshape(128, C, NPAIR * NCH))
        d = dict(w)
        d["xmP"] = np.ascontiguousarray(xmP)
        d["xl"] = pack_loss(x).astype(NPBF)
        d["ml"] = pack_loss(fm.astype(np.float32)).astype(NPBF)
        per_core.append(d)

    scale = np.std(x.astype(np.float64), axis=(0, 1), ddof=1) + 1e-8
    stats = {"inv_s2": 1.0 / (scale * scale), "msum": float(fm.sum())}
    return per_core, stats


def host_finalize(core_outs, stats):
    sf = np.sum([np.asarray(o, np.float64) for o in core_outs], axis=0)
    sf = sf[:64, 0] + sf[64:, 0]
    num = float(np.sum(sf * stats["inv_s2"]))
    den = max(stats["msum"], 1.0)
    return np.float32(num / den)


def build_program(T, phases="abc", repeats=1):
    assert T % C == 0
    NCH = T // C                  # 64 chunks/row
    NG = NPAIR * NCH              # 256 columns per parity group
    NCOLT = 2 * NG                # 512 total scan columns
    STEPS = W + C                 # 72
    Tp = T + C                    # 4160 = (NCH+1)*C
    NT = T // TILE_T
    NBLK = C // JB

    nc = bacc.Bacc("TRN2", target_bir_lowering=False, debug=False,
                   num_devices=NCORE)

    xmP = nc.dram_tensor("xmP", [NPAIR, 128, T], BF16, kind="ExternalInput").ap()
    xl = nc.dram_tensor("xl", [128, C, NG], BF16, kind="ExternalInput").ap()
    ml = nc.dram_tensor("ml", [128, C, NG], BF16, kind="ExternalInput").ap()
    stemW2 = nc.dram_tensor("stemW2", [128, 128], BF16, kind="ExternalInput").ap()
    stemB2 = nc.dram_tensor("stemB2", [128, 1], F32, kind="ExternalInput").ap()
    convW2 = nc.dram_tensor("convW2", [3, 128, 128], BF16, kind="ExternalInput").ap()
    convB2 = nc.dram_tensor("convB2", [128, 1], F32, kind="ExternalInput").ap()
    wihT = nc.dram_tensor("wihT", [3, DH, DG], BF16, kind="ExternalInput").ap()
    whhT = nc.dram_tensor("whhT", [3, DG, DG], BF16, kind="ExternalInput").ap()
    bR = nc.dram_tensor("bR", [DG, 1], F32, kind="ExternalInput").ap()
    bZ = nc.dram_tensor("bZ", [DG, 1], F32, kind="ExternalInput").ap()
    bIN = nc.dram_tensor("bIN", [DG, 1], F32, kind="ExternalInput").ap()
    bHN = nc.dram_tensor("bHN", [DG, 1], F32, kind="ExternalInput").ap()
    h1w = nc.dram_tensor("h1w", [DG, 128], BF16, kind="ExternalInput").ap()
    h1b = nc.dram_tensor("h1b", [128, 1], F32, kind="ExternalInput").ap()
    h2w = nc.dram_tensor("h2w", [128, 128], BF16, kind="ExternalInput").ap()
    h2b = nc.dram_tensor("h2b", [128, 1], F32, kind="ExternalInput").ap()
    h3w = nc.dram_tensor("h3w", [128, F], BF16, kind="ExternalInput").ap()
    h3b2 = nc.dram_tensor("h3b2", [128, 1], F32, kind="ExternalInput").ap()
    out = nc.dram_tensor("out", [128, 1], F32, kind="ExternalOutput").ap()

    with tile.TileContext(nc) as tc, ExitStack() as ctx:
        wpool = ctx.enter_context(tc.tile_pool(name="weights", bufs=1))
        spool = ctx.enter_context(tc.tile_pool(name="stats", bufs=1))
        bigpool = ctx.enter_context(tc.tile_pool(name="big", bufs=1))

        def wtile(shape, src, tag, dt=BF16):
            t = wpool.tile(shape, dt, tag=tag)
            nc.sync.dma_start(t[:], src)
            return t

        stemW_s = wtile([128, 128], stemW2[:], "w_stem")
        stemB_s = wtile([128, 1], stemB2[:], "w_stemb", F32)
        convW_s = [wtile([128, 128], convW2[dt], f"w_conv{dt}") for dt in range(3)]
        convB_s = wtile([128, 1], convB2[:], "w_convb", F32)
        wih_s = [wtile([DH, DG], wihT[g], f"w_wih{g}") for g in range(3)]
        whh_s = [wtile([DG, DG], whhT[g], f"w_whh{g}") for g in range(3)]
        bR_s = wtile([DG, 1], bR[:], "w_br", F32)
        bZ_s = wtile([DG, 1], bZ[:], "w_bz", F32)
        bIN_s = wtile([DG, 1], bIN[:], "w_bin", F32)
        bHN_s = wtile([DG, 1], bHN[:], "w_bhn", F32)
        h1w_s = wtile([DG, 128], h1w[:], "w_h1")
        h1b_s = wtile([128, 1], h1b[:], "w_h1b", F32)
        h2w_s = wtile([128, 128], h2w[:], "w_h2")
        h2b_s = wtile([128, 1], h2b[:], "w_h2b", F32)
        h3w_s = wtile([128, F], h3w[:], "w_h3")
        h3b2_s = wtile([128, 1], h3b2[:], "w_h3b", F32)

        zeros_s = wpool.tile([DG, NG], BF16)
        nc.vector.memset(zeros_s[:], 0.0)
        cmask = wpool.tile([DG, NPAIR, NCH], BF16)   # 0 on chunk-0 columns
        nc.vector.memset(cmask[:], 1.0)
        nc.vector.memset(cmask[:, :, 0:1], 0.0)

        convh = bigpool.tile([DH, B_C, Tp], BF16)    # unpacked padded conv out
        zbuf = bigpool.tile([DG, C, NCOLT], BF16)    # hidden states, scan order
        sf_cols = spool.tile([128, NBLK], F32)

        nc.vector.memset(convh[:, :, 0:W], 0.0)
        nc.vector.memset(convh[:, :, W + T:], 0.0)
        if "c" not in phases:
            nc.vector.memset(sf_cols[:], 0.0)

        # ================= Phase A: encoder =================
        if "a" in phases:
         with tc.tile_pool(name="enc_io", bufs=3) as eio, \
             tc.tile_pool(name="enc_row", bufs=2) as erow, \
             tc.tile_pool(name="enc_ps", bufs=3, space="PSUM") as eps:
           for _rep in range(repeats):
            for pair in range(NPAIR):
                hrow = erow.tile([128, T + 2], BF16, tag="hrow")
                nc.vector.memset(hrow[:, 0:1], 0.0)
                nc.vector.memset(hrow[:, T + 1:T + 2], 0.0)
                for it in range(NT):
                    xt = eio.tile([128, TILE_T], BF16, tag="xt")
                    nc.sync.dma_start(xt[:], xmP[pair, :, ts(it, TILE_T)])
                    ps = eps.tile([128, TILE_T], F32, tag="eps")
                    nc.tensor.matmul(ps[:], stemW_s[:], xt[:], start=True, stop=True)
                    nc.scalar.activation(
                        hrow[:, 1 + it * TILE_T:1 + (it + 1) * TILE_T], ps[:],
                        AF.Gelu, bias=stemB_s[:])
                for it in range(NT):
                    pc = eps.tile([128, TILE_T], F32, tag="eps")
                    for dt in range(3):
                        nc.tensor.matmul(
                            pc[:], convW_s[dt][:],
                            hrow[:, it * TILE_T + dt:it * TILE_T + dt + TILE_T],
                            start=(dt == 0), stop=(dt == 2))
                    ct = eio.tile([128, TILE_T], BF16, tag="ct")
                    nc.scalar.activation(ct[:], pc[:], AF.Gelu, bias=convB_s[:])
                    dsl = slice(W + it * TILE_T, W + (it + 1) * TILE_T)
                    nc.sync.dma_start(convh[:, 2 * pair, dsl], ct[0:64])
                    nc.sync.dma_start(convh[:, 2 * pair + 1, dsl], ct[64:128])

        # scan view: conv value for (row 2p+q, t=c*C+i-W) at [P, q, p, c, i]
        cview = convh[:].rearrange("P (p q) (c j) -> P q p c j", q=2, c=NCH + 1)

        # ================= Phase B: scan =================
        if "b" in phases:
         with tc.tile_pool(name="sc_ps", bufs=1, space="PSUM") as sps, \
             tc.tile_pool(name="sc_hn", bufs=2, space="PSUM") as shn, \
             tc.tile_pool(name="sc_sb", bufs=3) as ssb:

            def prefill(b):
                i0 = BS * b
                pre = sps.tile([DG, 3, BS, NCOLT], F32, tag="pre")
                for ib in range(BS):
                    i = i0 + ib
                    c0, jj = (0, i) if i < C else (1, i - C)
                    v = cview[:, :, :, c0:c0 + NCH, jj]
                    rv = v.rearrange("P q p c -> P q p c")
                    for g in range(3):
                        nc.tensor.matmul(pre[:, g, ib], wih_s[g][:], rv,
                                         start=True, stop=(g == 2),
                                         skip_group_check=True)
                return pre

            h_prev = {0: zeros_s[:], 1: zeros_s[:]}
            st = {}
            pre = prefill(0)

            def front(i, q):
                ib = i % BS
                qs = slice(q * NG, (q + 1) * NG)
                hp = h_prev[q]
                if i == W:
                    hm = ssb.tile([DG, NPAIR, NCH], BF16, tag=f"hm{q}")
                    nc.vector.tensor_mul(hm[:], hp.rearrange(
                        "d (p c) -> d p c", p=NPAIR), cmask[:])
                    hp = hm[:].rearrange("d p c -> d (p c)")
                    h_prev[q] = hp
                if q == 0:
                    st["hn"] = shn.tile([DG, 2, NG], F32, tag="hn", name="hn")
                hn = st["hn"]
                nc.tensor.matmul(pre[:, 1, ib, qs], whh_s[1][:], hp,
                                 start=False, stop=True, skip_group_check=True)
                z_s = ssb.tile([DG, NG], BF16, tag=f"z{q}")
                nc.scalar.activation(z_s[:], pre[:, 1, ib, qs], AF.Sigmoid,
                                     bias=bZ_s[:])
                nc.tensor.matmul(pre[:, 0, ib, qs], whh_s[0][:], hp,
                                 start=False, stop=True, skip_group_check=True)
                r_s = ssb.tile([DG, NG], BF16, tag=f"r{q}")
                nc.scalar.activation(r_s[:], pre[:, 0, ib, qs], AF.Sigmoid,
                                     bias=bR_s[:])
                nc.tensor.matmul(hn[:, q], whh_s[2][:], hp,
                                 start=True, stop=True, skip_group_check=True)
                t1 = ssb.tile([DG, NG], F32, tag=f"t1{q}")
                nc.vector.scalar_tensor_tensor(t1[:], hn[:, q], bHN_s[:],
                                               r_s[:], ALU.add, ALU.mult)
                narg = ssb.tile([DG, NG], F32, tag=f"na{q}")
                nc.vector.tensor_add(narg[:], t1[:], pre[:, 2, ib, qs])
                st[(q, "z")] = z_s
                st[(q, "narg")] = narg

            def back(i, q):
                z_s, narg = st[(q, "z")], st[(q, "narg")]
                n_s = ssb.tile([DG, NG], BF16, tag=f"n{q}")
                nc.scalar.activation(n_s[:], narg[:], AF.Tanh, bias=bIN_s[:])
                d_s = ssb.tile([DG, NG], BF16, tag=f"d{q}")
                nc.vector.tensor_sub(d_s[:], h_prev[q], n_s[:])
                v2 = ssb.tile([DG, NG], BF16, tag=f"v2{q}")
                nc.vector.tensor_mul(v2[:], z_s[:], d_s[:])
                if i >= W:
                    hdst = zbuf[:, i - W, q * NG:(q + 1) * NG]
                else:
                    hring = ssb.tile([DG, NG], BF16, tag=f"h{q}")
                    hdst = hring[:]
                nc.vector.tensor_add(hdst, n_s[:], v2[:])
                h_prev[q] = hdst

            for i in range(STEPS):
                front(i, 0)
                back(i, 0)
                front(i, 1)
                if i % BS == BS - 1 and i + 1 < STEPS:
                    pre = prefill((i + 1) // BS)
                back(i, 1)

        # ================= Phase C: head + loss =================
        if "c" in phases:
         with tc.tile_pool(name="hd_ps", bufs=2, space="PSUM") as hps, \
             tc.tile_pool(name="hd_sb", bufs=3) as hsb, \
             tc.tile_pool(name="ls_io", bufs=3) as lio:
            xlt = {}
            mlt = {}

            def stage_loss(jb):
                if jb * XLB >= C or jb in xlt:
                    return
                xt = lio.tile([128, XLB, NG], BF16, tag="xlt")
                mt = lio.tile([128, XLB, NG], BF16, tag="mlt")
                nc.sync.dma_start(xt[:], xl[:, ts(jb, XLB), :])
                nc.sync.dma_start(mt[:], ml[:, ts(jb, XLB), :])
                xlt[jb] = xt
                mlt[jb] = mt

            stage_loss(0)
            for blk in range(NBLK):
                j0 = blk * JB
                stage_loss(j0 // XLB + 1)
                p1 = hps.tile([128, JB, NCOLT], F32, tag="p12")
                for jj in range(JB):
                    nc.tensor.matmul(p1[:, jj], h1w_s[:], zbuf[:, j0 + jj, :],
                                     start=True, stop=True,
                                     skip_group_check=True)
                r1 = hsb.tile([128, JB, NCOLT], BF16, tag="r1")
                nc.scalar.activation(r1[:], p1[:], AF.Gelu, bias=h1b_s[:])
                p2 = hps.tile([128, JB, NCOLT], F32, tag="p12")
                for jj in range(JB):
                    nc.tensor.matmul(p2[:, jj], h2w_s[:], r1[:, jj],
                                     start=True, stop=True,
                                     skip_group_check=True)
                r2 = hsb.tile([128, JB, NCOLT], BF16, tag="r2")
                nc.scalar.activation(r2[:], p2[:], AF.Gelu, bias=h2b_s[:])
                p3 = hps.tile([128, JB, NG], F32, tag="p3")
                nc.tensor.matmul(p3[0:64], h3w_s[:], r2[:, :, 0:NG],
                                 start=True, stop=True, skip_group_check=True)
                nc.tensor.matmul(p3[64:128], h3w_s[:], r2[:, :, NG:NCOLT],
                                 start=True, stop=True, skip_group_check=True)
                xt, mt = xlt[j0 // XLB], mlt[j0 // XLB]
                jm = j0 % XLB
                diff = hsb.tile([128, JB, NG], F32, tag="diff")
                nc.vector.scalar_tensor_tensor(diff[:], p3[:], h3b2_s[:],
                                               xt[:, jm:jm + JB, :],
                                               ALU.add, ALU.subtract)
                dmm = hsb.tile([128, JB, NG], F32, tag="dmm")
                nc.vector.tensor_mul(dmm[:], diff[:], mt[:, jm:jm + JB, :])
                junk = hsb.tile([128, JB, NG], F32, tag="junk")
                if USE_TTR:
                    nc.vector.tensor_tensor_reduce(
                        junk[:], dmm[:], dmm[:], 1.0, 0.0, ALU.mult, ALU.add,
                        accum_out=sf_cols[:, blk:blk + 1])
                else:
                    nc.vector.tensor_mul(junk[:], dmm[:], dmm[:])
                    nc.vector.tensor_reduce(
                        sf_cols[:, blk:blk + 1],
                        junk[:].rearrange("p a b -> p (a b)"),
                        mybir.AxisListType.X, ALU.add)

        sf_out = spool.tile([128, 1], F32)
        nc.vector.tensor_reduce(sf_out[:], sf_cols[:],
                                mybir.AxisListType.X, ALU.add)
        nc.sync.dma_start(out[:], sf_out[:])

    nc.compile()
    return nc


_CACHE = {}


def kernel(**inputs):
    from concourse.bass_utils import run_bass_kernel_spmd

    T = int(np.asarray(inputs["x"]).shape[1])
    if "nc" not in _CACHE:
        _CACHE["nc"] = build_program(T)
    nc = _CACHE["nc"]
    per_core, stats = prep_inputs(inputs, T)
    res = run_bass_kernel_spmd(nc, per_core, list(range(NCORE))).results
    return host_finalize([r["out"] for r in res], stats)
